# revision 1
# baseline (speedup 1.0000x reference)
"""HarsanyiNet forward on 8 TRN2 NeuronCores (Bass/Tile).

Model (reference):
    harsanyi_block(x, v, fc):
        m = (v > 0)                                    # [O, I] mask
        delta = prod_i [ tanh(g*|x_i|) if m else 1 ]   # [B, O]
        h = relu((x @ (fc*m).T) * delta)
    y = h0 @ head0.T + h1 @ head1.T   (two blocks, h0 feeds block 1)

Key algebraic moves:
  * The [B, O, I] masked product becomes a matmul in log space:
        delta = exp(L @ m.T),  L[b,i] = log(tanh(g*|x[b,i]|))
    with log(tanh(y)) = ln(1-z) - ln(1+z), z = exp(-2*g*y), so the
    whole transcendental chain is {abs, exp, ln} — all in ONE ScalarE
    table set (natural_log_exp_and_others) -> a single table load.
  * Matmuls run on the bf16 PE path (4x the fp32 rate) with hi/lo
    split operands for fp32-grade accuracy.  The mask m is exact in
    bf16; fc and x are split on the host (w_hi = m*bf16_hi(fc) is
    exact because masking by 0/1 commutes with rounding); L is split
    on-device.  The z<=1-2^-24 clamp keeps Ln inputs positive, so
    every intermediate stays finite.
  * The serial DMA->abs->exp->min->ln->ln->sub->split chain is
    pipelined in two column halves so ScalarE and VectorE stages of
    half 0 overlap half 1.

Sharding: the output-hidden dim is split across the 8 cores, so each
core reads only 1/8 of v/fc per layer (~0.8 MB/core/launch instead of
16.4 MB replicated).  Layer 1 needs the full h0, which is bounced
through the host between two launches of the SAME compiled program
(an on-device AllGather costs ~80us in this environment, the host
bounce costs zero device time).  Partial head outputs are summed on
the host.

Layout: on-device tensors are feature-major [feature, batch]; the
1024-long feature dims are pre-split on the host into 8 chunk-major
blocks of 128 partitions, so every DMA is one dense [128, N] transfer
and every matmul operand slice is a natural column block.
"""
import sys

import numpy as np

sys.path.insert(0, "/opt/trn_rl_repo")

import ml_dtypes  # noqa: E402

from concourse import bacc, mybir, tile  # noqa: E402
from concourse.alu_op_type import AluOpType  # noqa: E402
from concourse.bass_utils import run_bass_kernel_spmd  # noqa: E402
from concourse.tile_rust import add_dep_helper  # noqa: E402


def _order(after, before, why):
    """Order-only scheduling edge: `after` runs after `before`."""
    add_dep_helper(getattr(after, "ins", after), getattr(before, "ins", before),
                   sync=False, reason=why)

B, NIN, HID, C = 64, 1024, 1024, 10
GAMMA = 100.0
N_CORES = 8
OSH = HID // N_CORES        # output-hidden rows per core (128)
KCH = NIN // 128            # contraction chunks (8)
KB = KCH * B                # activation columns, chunk-major (512)
KO = KCH * OSH              # weight columns, chunk-major (1024)
NH = 2                      # pipeline halves for the L chain
HB = KB // NH               # columns per half (256)
HCH = KCH // NH             # chunks per half (4)
# Upper clamp for z = exp(-2g|x|): keeps 1-z >= 2^-24 so Ln never sees 0
# (the reference's exact-zero delta becomes exp(-16.6)~3e-8 per factor,
# far below the output's scale).
LCLAMP = -30000.0
F32 = mybir.dt.float32
BF16 = mybir.dt.bfloat16
BF16_NP = ml_dtypes.bfloat16

PROFILE = {"enable": False, "trace_kwargs": {}, "runs": []}
_CACHE = {}


def _force_act_table_set(target="natural_log_exp_and_others"):
    """Make the act-table-load pass place every activation in `target`
    (it otherwise picks the first set per function, costing one ~2.7us
    table switch per transition Exp->Ln->Exp).  Indices of the table
    list are act_func_set_ids, so ordering is preserved and all other
    sets are emptied."""
    import concourse.bacc as bacc_mod
    from concourse.hw_specs import get_activation_tables as real_tabs

    def patched(arch):
        tabs = real_tabs(arch)
        return {name: (funcs if name == target else set())
                for name, funcs in tabs.items()}

    bacc_mod.get_activation_tables = patched


def _build():
    _force_act_table_set()
    nc = bacc.Bacc("TRN2", target_bir_lowering=False, debug=False,
                   num_devices=N_CORES, enable_asserts=False)
    xTf = nc.declare_dram_parameter("xTf", [128, KB], F32, isOutput=False)
    # bf16 hi/lo pairs packed side by side: [hi | lo]
    xhl = nc.declare_dram_parameter("xhl", [128, 2 * KB], BF16, isOutput=False)
    vT = nc.declare_dram_parameter("vT", [128, KO], BF16, isOutput=False)
    fhl = nc.declare_dram_parameter("fhl", [128, 2 * KO], BF16, isOutput=False)
    hdT = nc.declare_dram_parameter("hdT", [OSH, C], F32, isOutput=False)
    h_sh = nc.declare_dram_parameter("h_sh", [OSH, B], F32, isOutput=True)
    y_part = nc.declare_dram_parameter("y_part", [C, B], F32, isOutput=True)
    Act = mybir.ActivationFunctionType

    with tile.TileContext(nc) as tc:
        with (
            tc.tile_pool(name="sb", bufs=1) as sb,
            tc.tile_pool(name="ps", bufs=1, space="PSUM") as ps,
        ):
            # x (f32) arrives in NH column-halves so the L chain can start
            # on half 0 while half 1 is still in flight.
            xf = sb.tile([128, KB], F32)
            for hf in range(NH):
                nc.sync.dma_start(xf[:, hf * HB:(hf + 1) * HB],
                                  xTf[:, hf * HB:(hf + 1) * HB])
            # Weight DMAs ordered by first use: v (mask) -> fc_hi -> x
            # hi/lo (w_hi matmuls) -> fc_lo (w_lo matmuls, last 8 MMs).
            vt = sb.tile([128, KO], BF16)
            nc.sync.dma_start(vt[:], vT[:, :])
            fb = sb.tile([128, 2 * KO], BF16)
            nc.sync.dma_start(fb[:, :KO], fhl[:, :KO])
            xb = sb.tile([128, 2 * KB], BF16)
            nc.sync.dma_start(xb[:], xhl[:, :])
            nc.sync.dma_start(fb[:, KO:], fhl[:, KO:])
            hdt = sb.tile([OSH, C], F32)
            nc.sync.dma_start(hdt[:], hdT[:, :])

            # L = log(tanh(g*|x|)) = ln(1-z) - ln(1+z), z = exp(-2g|x|),
            # pipelined over NH column halves.  The small negative bias on
            # the Exp input keeps z strictly below 1 (so Ln(1-z) is finite
            # for x = 0); the -30000 clamp catches -inf if the LUT rounds
            # z up to 1 anyway.
            a = sb.tile([128, KB], F32)
            z = sb.tile([128, KB], F32)
            p = sb.tile([128, KB], F32)
            q = sb.tile([128, KB], F32)
            Lh = sb.tile([128, KB], BF16)
            Ll = sb.tile([128, KB], BF16)
            L = sb.tile([128, KB], F32)
            eps = sb.tile([128, 1], F32)
            nc.vector.memset(eps[:], -1e-6)
            m = sb.tile([128, KO], BF16)
            w = sb.tile([128, 2 * KO], BF16)
            S = ps.tile([OSH, B], F32)
            HL = ps.tile([OSH, B], F32)
            n_s = 2 * KCH
            i_s = 0
            s_last = None

            def s_mms(hf):
                nonlocal i_s, s_last
                for k in range(hf * HCH, (hf + 1) * HCH):
                    osl = slice(k * OSH, (k + 1) * OSH)
                    bsl = slice(k * B, (k + 1) * B)
                    for rhs in (Lh, Ll):
                        s_last = nc.tensor.matmul(S[:], m[:, osl],
                                                  rhs[:, bsl],
                                                  start=(i_s == 0),
                                                  stop=(i_s == n_s - 1))
                        i_s += 1

            subl = None
            for hf in range(NH):
                cs = slice(hf * HB, (hf + 1) * HB)
                nc.vector.scalar_tensor_tensor(a[:, cs], xf[:, cs], -1.0,
                                               xf[:, cs],
                                               op0=AluOpType.mult,
                                               op1=AluOpType.max)
                nc.scalar.activation(z[:, cs], a[:, cs], Act.Exp,
                                     scale=-2.0 * GAMMA, bias=eps[:])
                nc.scalar.activation(p[:, cs], z[:, cs], Act.Ln,
                                     bias=1.0, scale=-1.0)
                nc.scalar.activation(q[:, cs], z[:, cs], Act.Ln,
                                     bias=1.0, scale=1.0)
                if hf == 0:
                    # m = (v > 0) as 0/1 (v is exactly +-1): slot into the
                    # DVE stream while ScalarE works on ln, right before
                    # the first L ops so S matmuls can start early.
                    nc.vector.tensor_scalar_max(m[:], vt[:], 0.0)
                nc.vector.scalar_tensor_tensor(L[:, cs], p[:, cs], LCLAMP,
                                               q[:, cs],
                                               op0=AluOpType.max,
                                               op1=AluOpType.subtract)
                nc.vector.tensor_copy(Lh[:, cs], L[:, cs])
                subl = nc.vector.tensor_sub(Ll[:, cs], L[:, cs], Lh[:, cs])
                s_mms(hf)

            # w = fc * m.  The scheduler's cost model doesn't see DMA
            # latency and would hoist these (blocked on the fc DMA) ahead
            # of the ready L-chain ops on the in-order DVE; pin them after
            # the last L split.
            w0 = nc.vector.tensor_mul(w[:, :KO], m[:], fb[:, :KO])
            w1 = nc.vector.tensor_mul(w[:, KO:], m[:], fb[:, KO:])
            _order(w0, subl, "w after L splits (DVE head-of-line)")
            _order(w1, w0, "w_lo after w_hi")

            # HL matmuls last: they wait on the (late) fc DMA anyway, and
            # keeping them off the in-order PE queue lets S finish early.
            # HL += w_hi.T x_hi + w_hi.T x_lo + w_lo.T x_hi.  All 16 w_hi
            # matmuls first (they only need fc_hi, which lands before
            # fc_lo), then the 8 w_lo ones.
            n_hl = 3 * KCH
            i_hl = 0
            passes = [(slice(k * OSH, (k + 1) * OSH),
                       slice(k * B + off, (k + 1) * B + off))
                      for off in (0, KB) for k in range(KCH)]
            passes += [(slice(KO + k * OSH, KO + (k + 1) * OSH),
                        slice(k * B, (k + 1) * B)) for k in range(KCH)]
            for lsl, rsl in passes:
                mm = nc.tensor.matmul(HL[:], w[:, lsl], xb[:, rsl],
                                      start=(i_hl == 0),
                                      stop=(i_hl == n_hl - 1))
                if i_hl == 0:
                    _order(mm, s_last, "HL matmuls after S matmuls (PE)")
                i_hl += 1

            # h = relu(HL) * exp(S)   (= relu(HL*exp(S)) since exp(S) > 0;
            # the relu runs as soon as HL closes, in parallel with exp)
            d = sb.tile([OSH, B], F32)
            nc.scalar.activation(d[:], S[:], Act.Exp)
            hr = sb.tile([OSH, B], F32)
            nc.vector.tensor_scalar_max(hr[:], HL[:], 0.0)
            h = sb.tile([OSH, B], F32)
            nc.vector.tensor_mul(h[:], hr[:], d[:])
            nc.sync.dma_start(h_sh[:, :], h[:])

            # y_part[c,b] = sum_{o in shard} head[o,c]*h[o,b]  (fp32 PE)
            Y = ps.tile([C, B], F32)
            nc.tensor.matmul(Y[:], hdt[:, :], h[:], start=True, stop=True)
            yo = sb.tile([C, B], F32)
            nc.vector.tensor_copy(yo[:], Y[:])
            nc.sync.dma_start(y_part[:, :], yo[:])
    nc.compile()
    return nc


def _chunk_major(mat_t: np.ndarray) -> np.ndarray:
    """[1024, cols] -> [128, KCH*cols]: row block k lands at column
    offset k*cols, so partition dim is 128 and chunk k is a column
    slice."""
    rows, cols = mat_t.shape
    assert rows == KCH * 128
    return np.ascontiguousarray(
        mat_t.reshape(KCH, 128, cols).transpose(1, 0, 2).reshape(128, KCH * cols)
    )


def _split_hi_lo_packed(arr_f32: np.ndarray):
    hi = arr_f32.astype(BF16_NP)
    lo = (arr_f32 - hi.astype(np.float32)).astype(BF16_NP)
    return np.ascontiguousarray(np.concatenate([hi, lo], axis=1))


def _run_layer(nc, act, v, fc, head):
    """act: [B, 1024] layer input. Returns (h [B, HID], y_partial [C, B])."""
    xT = _chunk_major(np.ascontiguousarray(act.T.astype(np.float32)))
    xhl = _split_hi_lo_packed(xT)
    in_maps = []
    for c in range(N_CORES):
        sl = slice(c * OSH, (c + 1) * OSH)
        fT = _chunk_major(np.ascontiguousarray(fc[sl].T.astype(np.float32)))
        in_maps.append({
            "xTf": xT,
            "xhl": xhl,
            "vT": _chunk_major(np.ascontiguousarray(v[sl].T)).astype(BF16_NP),
            "fhl": _split_hi_lo_packed(fT),
            "hdT": np.ascontiguousarray(head[:, sl].T.astype(np.float32)),
        })
    kwargs = {}
    if PROFILE["enable"]:
        kwargs = {"trace": True, **PROFILE["trace_kwargs"]}
    res = run_bass_kernel_spmd(nc, in_maps, core_ids=list(range(N_CORES)),
                               **kwargs)
    if PROFILE["enable"]:
        PROFILE["runs"].append(res)
    hT = np.concatenate([res.results[c]["h_sh"] for c in range(N_CORES)],
                        axis=0)                      # [HID, B]
    y = np.zeros((C, B), np.float32)
    for c in range(N_CORES):
        y += res.results[c]["y_part"]
    return np.ascontiguousarray(hT.T), y


def kernel(x, v0, fc0, head0, v1, fc1, head1):
    nc = _CACHE.get("nc")
    if nc is None:
        nc = _CACHE["nc"] = _build()
    h0, yA = _run_layer(nc, np.asarray(x, np.float32), v0, fc0, head0)
    _, yB = _run_layer(nc, h0, v1, fc1, head1)
    return np.ascontiguousarray((yA + yB).T).astype(np.float32)



# revision 2
# speedup vs baseline: 1.1315x; 1.1315x over previous
"""HarsanyiNet forward on 8 TRN2 NeuronCores (Bass/Tile), fused single launch.

Model (reference):
    harsanyi_block(x, v, fc):
        m = (v > 0)                                    # [O, I] mask
        delta = prod_i [ tanh(g*|x_i|) if m else 1 ]   # [B, O]
        h = relu((x @ (fc*m).T) * delta)
    y = h0 @ head0.T + h1 @ head1.T   (two blocks, h0 feeds block 1)

Key structure:
  * delta in log space: delta = exp(L @ m.T), L = log(tanh(g*|x|)) =
    ln(1-z) - ln(1+z) with z = exp(-2g|x|) -> only {exp, ln} on ScalarE,
    one activation-table set, loaded once (warm op at kernel start).
  * ONE launch for both layers.  A previous 2-launch version spent
    ~16us/launch on fixed preamble + semaphore-teardown; fusing pays it
    once.  Layer 0 is computed IN FULL on every core (weights
    replicated) so layer 1 needs no cross-core gather of h0; layer 1 is
    sharded across cores by output-hidden chunk (128 rows/core).
  * Mask m and w = fc*m are folded on the host (static weight
    transforms); all x-dependent arithmetic runs on device in bf16
    matmuls with f32 psum/chains (rel err ~1e-3..1e-2 < 2e-2 gate).
  * The layer-0 weight stream (m0 2MB + w0 2MB per core) is the
    critical path; compute pipelines per 128-row output chunk behind
    the DMA stream: S0/HL0 matmuls -> delta0/h0 -> layer-1 L chain ->
    S1/HL1 partial accumulation, all chunk-by-chunk.  DMA issue cost
    (~0.6us per dma_start on the issuing engine) is split between
    GpSimd (m-stream) and Sync (w-stream).
  * SPMD trick: each core's weight blocks are ROTATED so that chunk
    slot k holds output-chunk (core+k)%8.  The program is identical on
    all cores; slot 0 is always "this core's own" head0 chunk, so the
    head0 partial matmul can run early.  Layer-1 contraction slots are
    rotated identically so slot k of h0 matches slot k of m1/w1.
  * Only output: y_part [C, B] f32 per core (head partials); host sums.

Layouts: feature-major [feature, batch] on device; 1024-long dims are
pre-split on the host into chunk-major blocks of 128 partitions.
"""
import sys

import numpy as np

sys.path.insert(0, "/opt/trn_rl_repo")

import ml_dtypes  # noqa: E402

from concourse import bacc, mybir, tile  # noqa: E402
from concourse.alu_op_type import AluOpType  # noqa: E402
from concourse.bass_utils import run_bass_kernel_spmd  # noqa: E402
from concourse.tile_rust import add_dep_helper  # noqa: E402


def _order(after, before, why):
    """Order-only scheduling edge: `after` runs after `before`."""
    add_dep_helper(getattr(after, "ins", after), getattr(before, "ins", before),
                   sync=False, reason=why)

B, NIN, HID, C = 64, 1024, 1024, 10
GAMMA = 100.0
N_CORES = 8
KCH = 8                     # 128-row chunks per 1024-long dim
OSH = 128                   # layer-1 output rows per core
KB = KCH * B                # activation columns, chunk-major (512)
KO0 = KCH * KCH * 128       # layer-0 full stationary cols (8192)
KO1 = KCH * 128             # layer-1 shard stationary cols (1024)
# Upper clamp applied to ln(1-z): keeps L finite if the Exp LUT rounds z
# up to 1 (the reference's exact-zero delta becomes a ~e-30000 factor).
LCLAMP = -30000.0
F32 = mybir.dt.float32
BF16 = mybir.dt.bfloat16
FP8 = mybir.dt.float8e4
BF16_NP = ml_dtypes.bfloat16
FP8_NP = ml_dtypes.float8_e4m3
M_FP8 = False               # masks as fp8e4 stationary operands
M_DT = FP8 if M_FP8 else BF16
M_NP = FP8_NP if M_FP8 else BF16_NP

PROFILE = {"enable": False, "trace_kwargs": {}, "runs": []}
_CACHE = {}


def _force_act_table_set(target="natural_log_exp_and_others"):
    """Make the act-table-load pass place every activation in `target`
    (it otherwise picks the first set per function, costing one ~2.7us
    table switch per Exp->Ln transition).  Indices of the table list are
    act_func_set_ids, so ordering is preserved and other sets are
    emptied."""
    import concourse.bacc as bacc_mod
    from concourse.hw_specs import get_activation_tables as real_tabs

    def patched(arch):
        tabs = real_tabs(arch)
        return {name: (funcs if name == target else set())
                for name, funcs in tabs.items()}

    bacc_mod.get_activation_tables = patched


def _build():
    _force_act_table_set()
    nc = bacc.Bacc("TRN2", target_bir_lowering=False, debug=False,
                   num_devices=N_CORES, enable_asserts=False)
    xT = nc.declare_dram_parameter("xT", [128, KB], F32, isOutput=False)
    m0T = nc.declare_dram_parameter("m0T", [128, KO0], M_DT, isOutput=False)
    w0T = nc.declare_dram_parameter("w0T", [128, KO0], BF16, isOutput=False)
    m1T = nc.declare_dram_parameter("m1T", [128, KO1], M_DT, isOutput=False)
    w1T = nc.declare_dram_parameter("w1T", [128, KO1], BF16, isOutput=False)
    hd0T = nc.declare_dram_parameter("hd0T", [128, C], F32, isOutput=False)
    hd1T = nc.declare_dram_parameter("hd1T", [128, C], F32, isOutput=False)
    y_part = nc.declare_dram_parameter("y_part", [C, B], F32, isOutput=True)
    Act = mybir.ActivationFunctionType

    with tile.TileContext(nc) as tc:
        with (
            tc.tile_pool(name="sb", bufs=1) as sb,
            tc.tile_pool(name="ps", bufs=1, space="PSUM") as ps,
        ):
            # ---------------- SBUF tiles
            xf = sb.tile([128, KB], F32)
            xb = sb.tile([128, KB], BF16)
            m0 = sb.tile([128, KO0], M_DT)
            w0 = sb.tile([128, KO0], BF16)
            m1 = sb.tile([128, KO1], M_DT)
            w1 = sb.tile([128, KO1], BF16)
            hd0 = sb.tile([128, C], F32)
            hd1 = sb.tile([128, C], F32)
            eps = sb.tile([128, 1], F32)
            warm_i = sb.tile([128, 1], F32)
            warm_o = sb.tile([128, 1], F32)
            a0 = sb.tile([128, KB], F32)
            z0 = sb.tile([128, KB], F32)
            p0 = sb.tile([128, KB], F32)
            q0 = sb.tile([128, KB], F32)
            L0f = sb.tile([128, KB], F32)
            L0 = sb.tile([128, KB], BF16)
            d0 = sb.tile([128, KB], F32)
            h0f = sb.tile([128, KB], F32)
            h0b = sb.tile([128, KB], BF16)
            z1 = sb.tile([128, KB], F32)
            p1 = sb.tile([128, KB], F32)
            q1 = sb.tile([128, KB], F32)
            L1f = sb.tile([128, KB], F32)
            L1 = sb.tile([128, KB], BF16)
            d1 = sb.tile([128, B], F32)
            h1f = sb.tile([128, B], F32)
            yo = sb.tile([C, B], F32)
            # ---------------- PSUM
            S0p = ps.tile([128, KB], F32)
            HL0p = ps.tile([128, KB], F32)
            S1p = ps.tile([128, B], F32)
            HL1p = ps.tile([128, B], F32)
            Yp = ps.tile([C, B], F32)

            # ---------------- DMA issue: gpsimd takes the m-stream, sync
            # the w-stream, interleaved per output-chunk block so compute
            # can trail the stream chunk-by-chunk.
            prev = {"g": None, "s": None}

            def dma(eng, key, dst, src):
                op = eng.dma_start(dst, src)
                if prev[key] is not None:
                    _order(op, prev[key], f"{key}-dma order")
                prev[key] = op
                return op

            d_x = dma(nc.sync, "s", xf[:], xT[:, :])
            d_m0 = []
            d_w0 = []
            d_m1 = d_w1 = d_h0 = d_h1 = None
            for ko in range(KCH):
                sl = slice(ko * 1024, (ko + 1) * 1024)
                d_m0.append(dma(nc.gpsimd, "g", m0[:, sl], m0T[:, sl]))
                d_w0.append(dma(nc.sync, "s", w0[:, sl], w0T[:, sl]))
                if ko == 1:
                    d_m1 = dma(nc.gpsimd, "g", m1[:], m1T[:, :])
                    d_w1 = dma(nc.sync, "s", w1[:], w1T[:, :])
                if ko == 2:
                    d_h0 = dma(nc.gpsimd, "g", hd0[:], hd0T[:, :])
                    d_h1 = dma(nc.sync, "s", hd1[:], hd1T[:, :])

            # ---------------- constants + act-table warm (forces the one
            # table load to run at kernel start, overlapping the x DMA)
            nc.vector.memset(eps[:], -1e-6)
            nc.vector.memset(warm_i[:], 0.0)
            warm = nc.scalar.activation(warm_o[:], warm_i[:], Act.Exp)

            # in-order engine queues: chain every op on its engine
            tail = {"sc": warm, "ve": None, "pe": None}

            def q(key, op):
                if tail[key] is not None:
                    _order(op, tail[key], f"{key} queue order")
                tail[key] = op
                return op

            def mm(out_ap, lhs_ap, rhs_ap, start, stop, skip=False):
                return q("pe", nc.tensor.matmul(out_ap, lhs_ap, rhs_ap,
                                                start=start, stop=stop,
                                                skip_group_check=skip))

            # ---------------- layer-0 L chain (x is small; single shot)
            # a0 = |x|; z0 = exp(-2g*a0 - 1e-6); L0 = max(ln(1-z0), CL) - ln(1+z0)
            q("ve", nc.vector.tensor_copy(xb[:], xf[:]))
            q("ve", nc.vector.scalar_tensor_tensor(
                a0[:], xf[:], -1.0, xf[:],
                op0=AluOpType.mult, op1=AluOpType.max))
            q("sc", nc.scalar.activation(z0[:], a0[:], Act.Exp,
                                         scale=-2.0 * GAMMA, bias=eps[:]))
            q("sc", nc.scalar.activation(p0[:], z0[:], Act.Ln,
                                         bias=1.0, scale=-1.0))
            q("sc", nc.scalar.activation(q0[:], z0[:], Act.Ln,
                                         bias=1.0, scale=1.0))
            q("ve", nc.vector.scalar_tensor_tensor(
                L0f[:], p0[:], LCLAMP, q0[:],
                op0=AluOpType.max, op1=AluOpType.subtract))
            q("ve", nc.vector.tensor_copy(L0[:], L0f[:]))

            # ---------------- per-chunk pipeline
            # slot ko: S0/HL0 matmuls for output-chunk slot ko, then its
            # delta0/h0/L1 chain; layer-1 partial accumulation for slot
            # ko-1 goes after slot ko's matmuls so the in-order PE queue
            # never waits on a chain that is still running.
            def l1_partial(j):
                lsl = slice(j * 128, (j + 1) * 128)
                bsj = slice(j * B, (j + 1) * B)
                mm(S1p[:], m1[:, lsl], L1[:, bsj],
                   start=(j == 0), stop=(j == KCH - 1), skip=True)
                mm(HL1p[:], w1[:, lsl], h0b[:, bsj],
                   start=(j == 0), stop=(j == KCH - 1), skip=True)
                if j == 0:
                    # head0 partial for this core's own chunk (slot 0)
                    mm(Yp[:], hd0[:, :], h0f[:, 0:B],
                       start=True, stop=False, skip=True)

            for ko in range(KCH):
                bs = slice(ko * B, (ko + 1) * B)
                for ki in range(KCH):
                    lsl = slice(ko * 1024 + ki * 128, ko * 1024 + (ki + 1) * 128)
                    rsl = slice(ki * B, (ki + 1) * B)
                    mm(S0p[:, bs], m0[:, lsl], L0[:, rsl],
                       start=(ki == 0), stop=(ki == KCH - 1))
                for ki in range(KCH):
                    lsl = slice(ko * 1024 + ki * 128, ko * 1024 + (ki + 1) * 128)
                    rsl = slice(ki * B, (ki + 1) * B)
                    mm(HL0p[:, bs], w0[:, lsl], xb[:, rsl],
                       start=(ki == 0), stop=(ki == KCH - 1))
                if ko >= 1:
                    l1_partial(ko - 1)
                # chunk chain: delta0 -> h0 -> L1
                q("sc", nc.scalar.activation(d0[:, bs], S0p[:, bs], Act.Exp))
                q("ve", nc.vector.scalar_tensor_tensor(
                    h0f[:, bs], HL0p[:, bs], 0.0, d0[:, bs],
                    op0=AluOpType.max, op1=AluOpType.mult))
                q("ve", nc.vector.tensor_copy(h0b[:, bs], h0f[:, bs]))
                q("sc", nc.scalar.activation(z1[:, bs], h0f[:, bs], Act.Exp,
                                             scale=-2.0 * GAMMA, bias=eps[:]))
                q("sc", nc.scalar.activation(p1[:, bs], z1[:, bs], Act.Ln,
                                             bias=1.0, scale=-1.0))
                q("sc", nc.scalar.activation(q1[:, bs], z1[:, bs], Act.Ln,
                                             bias=1.0, scale=1.0))
                q("ve", nc.vector.scalar_tensor_tensor(
                    L1f[:, bs], p1[:, bs], LCLAMP, q1[:, bs],
                    op0=AluOpType.max, op1=AluOpType.subtract))
                q("ve", nc.vector.tensor_copy(L1[:, bs], L1f[:, bs]))
            l1_partial(KCH - 1)

            # ---------------- finale: h1 = relu(HL1)*exp(S1); y partial
            q("sc", nc.scalar.activation(d1[:], S1p[:], Act.Exp))
            q("ve", nc.vector.scalar_tensor_tensor(
                h1f[:], HL1p[:], 0.0, d1[:],
                op0=AluOpType.max, op1=AluOpType.mult))
            mm(Yp[:], hd1[:, :], h1f[:], start=False, stop=True, skip=True)
            q("ve", nc.vector.tensor_copy(yo[:], Yp[:]))
            dma(nc.sync, "s", y_part[:, :], yo[:])
    nc.compile()
    return nc


def _prep(x, v0, fc0, head0, v1, fc1, head1):
    """Host-side weight preprocessing -> per-core in_maps.

    Per core c, output-chunk slot k holds layer-0 output chunk
    (c+k)%8; layer-1 contraction slot k is rotated identically."""
    m0 = (np.asarray(v0) > 0).astype(np.float32)
    w0 = np.asarray(fc0, np.float32) * m0
    m1 = (np.asarray(v1) > 0).astype(np.float32)
    w1 = np.asarray(fc1, np.float32) * m1
    xT = np.asarray(x, np.float32).T                      # [1024, 64]
    xc = np.ascontiguousarray(
        xT.reshape(KCH, 128, B).transpose(1, 0, 2).reshape(128, KB))
    # [ki, ip, oc, op] blocks of the transposed layer-0 weights
    m0blk = m0.T.reshape(KCH, 128, KCH, 128)
    w0blk = w0.T.reshape(KCH, 128, KCH, 128)
    head0 = np.asarray(head0, np.float32)
    head1 = np.asarray(head1, np.float32)
    in_maps = []
    for c in range(N_CORES):
        perm = [(c + k) % KCH for k in range(KCH)]
        # slot-major k, then ki, then op: [ip, k, ki, op] -> [128, 8192]
        m0c = np.ascontiguousarray(
            m0blk[:, :, perm, :].transpose(1, 2, 0, 3).reshape(128, KO0)
        ).astype(M_NP)
        w0c = np.ascontiguousarray(
            w0blk[:, :, perm, :].transpose(1, 2, 0, 3).reshape(128, KO0)
        ).astype(BF16_NP)
        sl = slice(c * OSH, (c + 1) * OSH)
        m1t = m1[sl].T.reshape(KCH, 128, OSH)             # [ic, ip, o]
        w1t = w1[sl].T.reshape(KCH, 128, OSH)
        m1c = np.ascontiguousarray(
            m1t[perm].transpose(1, 0, 2).reshape(128, KO1)).astype(M_NP)
        w1c = np.ascontiguousarray(
            w1t[perm].transpose(1, 0, 2).reshape(128, KO1)).astype(BF16_NP)
        in_maps.append({
            "xT": xc,
            "m0T": m0c,
            "w0T": w0c,
            "m1T": m1c,
            "w1T": w1c,
            "hd0T": np.ascontiguousarray(head0[:, sl].T),
            "hd1T": np.ascontiguousarray(head1[:, sl].T),
        })
    return in_maps


def kernel(x, v0, fc0, head0, v1, fc1, head1):
    nc = _CACHE.get("nc")
    if nc is None:
        nc = _CACHE["nc"] = _build()
    in_maps = _prep(x, v0, fc0, head0, v1, fc1, head1)
    kwargs = {}
    if PROFILE["enable"]:
        kwargs = {"trace": True, **PROFILE["trace_kwargs"]}
    res = run_bass_kernel_spmd(nc, in_maps, core_ids=list(range(N_CORES)),
                               **kwargs)
    if PROFILE["enable"]:
        PROFILE["runs"].append(res)
    y = np.zeros((C, B), np.float32)
    for c in range(N_CORES):
        y += res.results[c]["y_part"]
    return np.ascontiguousarray(y.T).astype(np.float32)


# revision 4
# speedup vs baseline: 1.2616x; 1.1150x over previous
"""HarsanyiNet forward on 8 TRN2 NeuronCores (Bass/Tile), fused single launch.

Model (reference):
    harsanyi_block(x, v, fc):
        m = (v > 0)                                    # [O, I] mask
        delta = prod_i [ tanh(g*|x_i|) if m else 1 ]   # [B, O]
        h = relu((x @ (fc*m).T) * delta)
    y = h0 @ head0.T + h1 @ head1.T   (two blocks, h0 feeds block 1)

Key structure:
  * delta in log space: delta = exp(L @ m.T), L = log(tanh(g*|x|)) =
    ln(1-z) - ln(1+z) with z = exp(-2g|x|) -> only {exp, ln} on ScalarE,
    one activation-table set, loaded once (warm op at kernel start).
  * ONE launch for both layers.  A previous 2-launch version spent
    ~16us/launch on fixed preamble + semaphore-teardown; fusing pays it
    once.  Layer 0 is computed IN FULL on every core (weights
    replicated) so layer 1 needs no cross-core gather of h0; layer 1 is
    sharded across cores by output-hidden chunk (128 rows/core).
  * Mask m and w = fc*m are folded on the host (static weight
    transforms); all x-dependent arithmetic runs on device in bf16
    matmuls with f32 psum/chains (rel err ~1e-3..1e-2 < 2e-2 gate).
  * The layer-0 weight stream (m0 2MB + w0 2MB per core) is the
    critical path; compute pipelines per 128-row output chunk behind
    the DMA stream: S0/HL0 matmuls -> delta0/h0 -> layer-1 L chain ->
    S1/HL1 partial accumulation, all chunk-by-chunk.  DMA issue cost
    (~0.6us per dma_start on the issuing engine) is split between
    GpSimd (m-stream) and Sync (w-stream).
  * SPMD trick: each core's weight blocks are ROTATED so that chunk
    slot k holds output-chunk (core+k)%8.  The program is identical on
    all cores; slot 0 is always "this core's own" head0 chunk, so the
    head0 partial matmul can run early.  Layer-1 contraction slots are
    rotated identically so slot k of h0 matches slot k of m1/w1.
  * Only output: y_part [C, B] f32 per core (head partials); host sums.

Layouts: feature-major [feature, batch] on device; 1024-long dims are
pre-split on the host into chunk-major blocks of 128 partitions.
"""
import sys

import numpy as np

sys.path.insert(0, "/opt/trn_rl_repo")

import ml_dtypes  # noqa: E402

from concourse import bacc, mybir, tile  # noqa: E402
from concourse.alu_op_type import AluOpType  # noqa: E402
from concourse.bass_utils import run_bass_kernel_spmd  # noqa: E402
from concourse.tile_rust import add_dep_helper  # noqa: E402


def _order(after, before, why):
    """Order-only scheduling edge: `after` runs after `before`."""
    add_dep_helper(getattr(after, "ins", after), getattr(before, "ins", before),
                   sync=False, reason=why)

B, NIN, HID, C = 64, 1024, 1024, 10
GAMMA = 100.0
N_CORES = 8
KCH = 8                     # 128-row chunks per 1024-long dim
OSH = 128                   # layer-1 output rows per core
KB = KCH * B                # activation columns, chunk-major (512)
KO0 = KCH * KCH * 128       # layer-0 full stationary cols (8192)
KO1 = KCH * 128             # layer-1 shard stationary cols (1024)
# Upper clamp applied to ln(1-z): keeps L finite if the Exp LUT rounds z
# up to 1 (the reference's exact-zero delta becomes a ~e-30000 factor).
LCLAMP = -30000.0
F32 = mybir.dt.float32
BF16 = mybir.dt.bfloat16
FP8 = mybir.dt.float8e4
BF16_NP = ml_dtypes.bfloat16
FP8_NP = ml_dtypes.float8_e4m3
M_FP8 = False               # masks as fp8e4 stationary operands
M_DT = FP8 if M_FP8 else BF16
M_NP = FP8_NP if M_FP8 else BF16_NP

PROFILE = {"enable": False, "trace_kwargs": {}, "runs": []}
_CACHE = {}


def _force_act_table_set(target="natural_log_exp_and_others"):
    """Make the act-table-load pass place every activation in `target`
    (it otherwise picks the first set per function, costing one ~2.7us
    table switch per Exp->Ln transition).  Indices of the table list are
    act_func_set_ids, so ordering is preserved and other sets are
    emptied."""
    import concourse.bacc as bacc_mod
    from concourse.hw_specs import get_activation_tables as real_tabs

    def patched(arch):
        tabs = real_tabs(arch)
        return {name: (funcs if name == target else set())
                for name, funcs in tabs.items()}

    bacc_mod.get_activation_tables = patched


def _build():
    _force_act_table_set()
    nc = bacc.Bacc("TRN2", target_bir_lowering=False, debug=False,
                   num_devices=N_CORES, enable_asserts=False)
    xT = nc.declare_dram_parameter("xT", [128, KB], F32, isOutput=False)
    m0T = nc.declare_dram_parameter("m0T", [128, KO0], M_DT, isOutput=False)
    w0T = nc.declare_dram_parameter("w0T", [128, KO0], BF16, isOutput=False)
    m1T = nc.declare_dram_parameter("m1T", [128, KO1], M_DT, isOutput=False)
    w1T = nc.declare_dram_parameter("w1T", [128, KO1], BF16, isOutput=False)
    hd0T = nc.declare_dram_parameter("hd0T", [128, C], F32, isOutput=False)
    hd1T = nc.declare_dram_parameter("hd1T", [128, C], F32, isOutput=False)
    y_part = nc.declare_dram_parameter("y_part", [C, B], F32, isOutput=True)
    Act = mybir.ActivationFunctionType

    with tile.TileContext(nc) as tc:
        with (
            tc.tile_pool(name="sb", bufs=1) as sb,
            tc.tile_pool(name="ps", bufs=1, space="PSUM") as ps,
        ):
            # ---------------- SBUF tiles
            xf = sb.tile([128, KB], F32)
            xb = sb.tile([128, KB], BF16)
            m0 = sb.tile([128, KO0], M_DT)
            w0 = sb.tile([128, KO0], BF16)
            m1 = sb.tile([128, KO1], M_DT)
            w1 = sb.tile([128, KO1], BF16)
            hd0 = sb.tile([128, C], F32)
            hd1 = sb.tile([128, C], F32)
            eps = sb.tile([128, 1], F32)
            warm_i = sb.tile([128, 1], F32)
            warm_o = sb.tile([128, 1], F32)
            a0 = sb.tile([128, KB], F32)
            z0 = sb.tile([128, KB], F32)
            p0 = sb.tile([128, KB], F32)
            q0 = sb.tile([128, KB], F32)
            L0f = sb.tile([128, KB], F32)
            L0 = sb.tile([128, KB], BF16)
            d0 = sb.tile([128, KB], F32)
            h0f = sb.tile([128, KB], F32)
            h0b = sb.tile([128, KB], BF16)
            z1 = sb.tile([128, KB], F32)
            p1 = sb.tile([128, KB], F32)
            q1 = sb.tile([128, KB], F32)
            L1f = sb.tile([128, KB], F32)
            L1 = sb.tile([128, KB], BF16)
            d1 = sb.tile([128, B], F32)
            h1f = sb.tile([128, B], F32)
            yo = sb.tile([C, B], F32)
            # ---------------- PSUM
            S0p = ps.tile([128, KB], F32)
            HL0p = ps.tile([128, KB], F32)
            S1p = ps.tile([128, B], F32)
            HL1p = ps.tile([128, B], F32)
            Yp = ps.tile([C, B], F32)

            # ---------------- DMA issue: gpsimd takes the m-stream, sync
            # the w-stream.  Few, WIDE calls: descriptor count per call is
            # the 128 partition rows regardless of width, so wide rows
            # amortize both the ~10ns/row issue cost and the ~60ns/desc
            # ring overhead.  Halves (4096 cols) keep a 2-stage pipeline.
            prev = {"g": None, "s": None}

            def dma(eng, key, dst, src):
                op = eng.dma_start(dst, src)
                if prev[key] is not None:
                    _order(op, prev[key], f"{key}-dma order")
                prev[key] = op
                return op

            HALF = KO0 // 2
            d_x = dma(nc.sync, "s", xf[:], xT[:, :])
            d_m0 = [dma(nc.gpsimd, "g", m0[:, :HALF], m0T[:, :HALF])]
            d_w0 = [dma(nc.sync, "s", w0[:, :HALF], w0T[:, :HALF])]
            d_m1 = dma(nc.gpsimd, "g", m1[:], m1T[:, :])
            d_m0.append(dma(nc.gpsimd, "g", m0[:, HALF:], m0T[:, HALF:]))
            d_w1 = dma(nc.sync, "s", w1[:], w1T[:, :])
            d_w0.append(dma(nc.sync, "s", w0[:, HALF:], w0T[:, HALF:]))
            d_h0 = dma(nc.gpsimd, "g", hd0[:], hd0T[:, :])
            d_h1 = dma(nc.sync, "s", hd1[:], hd1T[:, :])

            # ---------------- constants + act-table warm (forces the one
            # table load to run at kernel start, overlapping the x DMA)
            nc.vector.memset(eps[:], -1e-6)
            nc.vector.memset(warm_i[:], 0.0)
            warm = nc.scalar.activation(warm_o[:], warm_i[:], Act.Exp)

            # in-order engine queues: chain every op on its engine
            tail = {"sc": warm, "ve": None, "pe": None}

            def q(key, op):
                if tail[key] is not None:
                    _order(op, tail[key], f"{key} queue order")
                tail[key] = op
                return op

            def mm(out_ap, lhs_ap, rhs_ap, start, stop, skip=False):
                return q("pe", nc.tensor.matmul(out_ap, lhs_ap, rhs_ap,
                                                start=start, stop=stop,
                                                skip_group_check=skip))

            # ---------------- layer-0 L chain (x is small; single shot)
            # a0 = |x|; z0 = exp(-2g*a0 - 1e-6); L0 = max(ln(1-z0), CL) - ln(1+z0)
            q("ve", nc.vector.tensor_copy(xb[:], xf[:]))
            q("ve", nc.vector.scalar_tensor_tensor(
                a0[:], xf[:], -1.0, xf[:],
                op0=AluOpType.mult, op1=AluOpType.max))
            q("sc", nc.scalar.activation(z0[:], a0[:], Act.Exp,
                                         scale=-2.0 * GAMMA, bias=eps[:]))
            q("sc", nc.scalar.activation(p0[:], z0[:], Act.Ln,
                                         bias=1.0, scale=-1.0))
            q("sc", nc.scalar.activation(q0[:], z0[:], Act.Ln,
                                         bias=1.0, scale=1.0))
            q("ve", nc.vector.scalar_tensor_tensor(
                L0f[:], p0[:], LCLAMP, q0[:],
                op0=AluOpType.max, op1=AluOpType.subtract))
            q("ve", nc.vector.tensor_copy(L0[:], L0f[:]))

            # ---------------- layer-0 matmul sweep + trailing chains
            # PE runs the full S0/HL0 sweep uninterrupted (layer-1 matmuls
            # wait until the end; they are only ~1.1us and would stall the
            # in-order PE queue mid-stream waiting on the chains).  The
            # delta0/h0/L1 chains trail in 2-chunk steps on Scalar/Vector.
            def chain_step(g):                 # chunks 2g, 2g+1
                bs = slice(2 * g * B, (2 * g + 2) * B)
                q("sc", nc.scalar.activation(d0[:, bs], S0p[:, bs], Act.Exp))
                q("ve", nc.vector.scalar_tensor_tensor(
                    h0f[:, bs], HL0p[:, bs], 0.0, d0[:, bs],
                    op0=AluOpType.max, op1=AluOpType.mult))
                q("ve", nc.vector.tensor_copy(h0b[:, bs], h0f[:, bs]))
                q("sc", nc.scalar.activation(z1[:, bs], h0f[:, bs], Act.Exp,
                                             scale=-2.0 * GAMMA, bias=eps[:]))
                q("sc", nc.scalar.activation(p1[:, bs], z1[:, bs], Act.Ln,
                                             bias=1.0, scale=-1.0))
                q("sc", nc.scalar.activation(q1[:, bs], z1[:, bs], Act.Ln,
                                             bias=1.0, scale=1.0))
                q("ve", nc.vector.scalar_tensor_tensor(
                    L1f[:, bs], p1[:, bs], LCLAMP, q1[:, bs],
                    op0=AluOpType.max, op1=AluOpType.subtract))
                q("ve", nc.vector.tensor_copy(L1[:, bs], L1f[:, bs]))

            for ko in range(KCH):
                bs = slice(ko * B, (ko + 1) * B)
                for ki in range(KCH):
                    lsl = slice(ko * 1024 + ki * 128, ko * 1024 + (ki + 1) * 128)
                    rsl = slice(ki * B, (ki + 1) * B)
                    mm(S0p[:, bs], m0[:, lsl], L0[:, rsl],
                       start=(ki == 0), stop=(ki == KCH - 1))
                for ki in range(KCH):
                    lsl = slice(ko * 1024 + ki * 128, ko * 1024 + (ki + 1) * 128)
                    rsl = slice(ki * B, (ki + 1) * B)
                    mm(HL0p[:, bs], w0[:, lsl], xb[:, rsl],
                       start=(ki == 0), stop=(ki == KCH - 1))
                if ko % 2 == 1:
                    chain_step(ko // 2)

            # ---------------- layer-1 + head matmuls, then finale
            for j in range(KCH):
                lsl = slice(j * 128, (j + 1) * 128)
                bsj = slice(j * B, (j + 1) * B)
                mm(S1p[:], m1[:, lsl], L1[:, bsj],
                   start=(j == 0), stop=(j == KCH - 1), skip=True)
                mm(HL1p[:], w1[:, lsl], h0b[:, bsj],
                   start=(j == 0), stop=(j == KCH - 1), skip=True)
            # head0 partial for this core's own chunk (slot 0)
            mm(Yp[:], hd0[:, :], h0f[:, 0:B], start=True, stop=False, skip=True)
            q("sc", nc.scalar.activation(d1[:], S1p[:], Act.Exp))
            q("ve", nc.vector.scalar_tensor_tensor(
                h1f[:], HL1p[:], 0.0, d1[:],
                op0=AluOpType.max, op1=AluOpType.mult))
            mm(Yp[:], hd1[:, :], h1f[:], start=False, stop=True, skip=True)
            q("ve", nc.vector.tensor_copy(yo[:], Yp[:]))
            dma(nc.sync, "s", y_part[:, :], yo[:])
    nc.compile()
    return nc


def _prep(x, v0, fc0, head0, v1, fc1, head1):
    """Host-side weight preprocessing -> per-core in_maps.

    Per core c, output-chunk slot k holds layer-0 output chunk
    (c+k)%8; layer-1 contraction slot k is rotated identically."""
    m0 = (np.asarray(v0) > 0).astype(np.float32)
    w0 = np.asarray(fc0, np.float32) * m0
    m1 = (np.asarray(v1) > 0).astype(np.float32)
    w1 = np.asarray(fc1, np.float32) * m1
    xT = np.asarray(x, np.float32).T                      # [1024, 64]
    xc = np.ascontiguousarray(
        xT.reshape(KCH, 128, B).transpose(1, 0, 2).reshape(128, KB))
    # [ki, ip, oc, op] blocks of the transposed layer-0 weights
    m0blk = m0.T.reshape(KCH, 128, KCH, 128)
    w0blk = w0.T.reshape(KCH, 128, KCH, 128)
    head0 = np.asarray(head0, np.float32)
    head1 = np.asarray(head1, np.float32)
    in_maps = []
    for c in range(N_CORES):
        perm = [(c + k) % KCH for k in range(KCH)]
        # slot-major k, then ki, then op: [ip, k, ki, op] -> [128, 8192]
        m0c = np.ascontiguousarray(
            m0blk[:, :, perm, :].transpose(1, 2, 0, 3).reshape(128, KO0)
        ).astype(M_NP)
        w0c = np.ascontiguousarray(
            w0blk[:, :, perm, :].transpose(1, 2, 0, 3).reshape(128, KO0)
        ).astype(BF16_NP)
        sl = slice(c * OSH, (c + 1) * OSH)
        m1t = m1[sl].T.reshape(KCH, 128, OSH)             # [ic, ip, o]
        w1t = w1[sl].T.reshape(KCH, 128, OSH)
        m1c = np.ascontiguousarray(
            m1t[perm].transpose(1, 0, 2).reshape(128, KO1)).astype(M_NP)
        w1c = np.ascontiguousarray(
            w1t[perm].transpose(1, 0, 2).reshape(128, KO1)).astype(BF16_NP)
        in_maps.append({
            "xT": xc,
            "m0T": m0c,
            "w0T": w0c,
            "m1T": m1c,
            "w1T": w1c,
            "hd0T": np.ascontiguousarray(head0[:, sl].T),
            "hd1T": np.ascontiguousarray(head1[:, sl].T),
        })
    return in_maps


def kernel(x, v0, fc0, head0, v1, fc1, head1):
    nc = _CACHE.get("nc")
    if nc is None:
        nc = _CACHE["nc"] = _build()
    in_maps = _prep(x, v0, fc0, head0, v1, fc1, head1)
    kwargs = {}
    if PROFILE["enable"]:
        kwargs = {"trace": True, **PROFILE["trace_kwargs"]}
    res = run_bass_kernel_spmd(nc, in_maps, core_ids=list(range(N_CORES)),
                               **kwargs)
    if PROFILE["enable"]:
        PROFILE["runs"].append(res)
    y = np.zeros((C, B), np.float32)
    for c in range(N_CORES):
        y += res.results[c]["y_part"]
    return np.ascontiguousarray(y.T).astype(np.float32)


# revision 8
# speedup vs baseline: 1.2769x; 1.0121x over previous
"""HarsanyiNet forward on 8 TRN2 NeuronCores (Bass/Tile), fused single launch.

Model (reference):
    harsanyi_block(x, v, fc):
        m = (v > 0)                                    # [O, I] mask
        delta = prod_i [ tanh(g*|x_i|) if m else 1 ]   # [B, O]
        h = relu((x @ (fc*m).T) * delta)
    y = h0 @ head0.T + h1 @ head1.T   (two blocks, h0 feeds block 1)

Key structure:
  * delta in log space: delta = exp(L @ m.T), L = log(tanh(g*|x|)) =
    ln(1-z) - ln(1+z) with z = exp(-2g|x|) -> only {exp, ln} on ScalarE,
    one activation-table set, loaded once (warm op at kernel start).
  * ONE launch for both layers.  A previous 2-launch version spent
    ~16us/launch on fixed preamble + semaphore-teardown; fusing pays it
    once.  Layer 0 is computed IN FULL on every core (weights
    replicated) so layer 1 needs no cross-core gather of h0; layer 1 is
    sharded across cores by output-hidden chunk (128 rows/core).
  * Mask m and w = fc*m are folded on the host (static weight
    transforms); all x-dependent arithmetic runs on device in bf16
    matmuls with f32 psum/chains (rel err ~1e-3..1e-2 < 2e-2 gate).
  * The layer-0 weight stream (m0 2MB + w0 2MB per core) is the
    critical path; compute pipelines per 128-row output chunk behind
    the DMA stream: S0/HL0 matmuls -> delta0/h0 -> layer-1 L chain ->
    S1/HL1 partial accumulation, all chunk-by-chunk.  DMA issue cost
    (~0.6us per dma_start on the issuing engine) is split between
    GpSimd (m-stream) and Sync (w-stream).
  * SPMD trick: each core's weight blocks are ROTATED so that chunk
    slot k holds output-chunk (core+k)%8.  The program is identical on
    all cores; slot 0 is always "this core's own" head0 chunk, so the
    head0 partial matmul can run early.  Layer-1 contraction slots are
    rotated identically so slot k of h0 matches slot k of m1/w1.
  * Only output: y_part [C, B] f32 per core (head partials); host sums.

Layouts: feature-major [feature, batch] on device; 1024-long dims are
pre-split on the host into chunk-major blocks of 128 partitions.
"""
import sys

import numpy as np

sys.path.insert(0, "/opt/trn_rl_repo")

import ml_dtypes  # noqa: E402

from concourse import bacc, mybir, tile  # noqa: E402
from concourse.alu_op_type import AluOpType  # noqa: E402
from concourse.bass_utils import run_bass_kernel_spmd  # noqa: E402
from concourse.tile_rust import add_dep_helper  # noqa: E402


def _order(after, before, why):
    """Order-only scheduling edge: `after` runs after `before`."""
    add_dep_helper(getattr(after, "ins", after), getattr(before, "ins", before),
                   sync=False, reason=why)

B, NIN, HID, C = 64, 1024, 1024, 10
GAMMA = 100.0
N_CORES = 8
KCH = 8                     # 128-row chunks per 1024-long dim
OSH = 128                   # layer-1 output rows per core
KB = KCH * B                # activation columns, chunk-major (512)
KO0 = KCH * KCH * 128       # layer-0 full stationary cols (8192)
KO1 = KCH * 128             # layer-1 shard stationary cols (1024)
# Upper clamp applied to ln(1-z): keeps L finite if the Exp LUT rounds z
# up to 1 (the reference's exact-zero delta becomes a ~e-30000 factor).
LCLAMP = -30000.0
F32 = mybir.dt.float32
BF16 = mybir.dt.bfloat16
FP8 = mybir.dt.float8e4
BF16_NP = ml_dtypes.bfloat16
FP8_NP = ml_dtypes.float8_e4m3
M_FP8 = True                # masks as fp8e4 stationary operands
M_DT = FP8 if M_FP8 else BF16
M_NP = FP8_NP if M_FP8 else BF16_NP

PROFILE = {"enable": False, "trace_kwargs": {}, "runs": []}
_CACHE = {}


def _force_act_table_set(target="natural_log_exp_and_others"):
    """Make the act-table-load pass place every activation in `target`
    (it otherwise picks the first set per function, costing one ~2.7us
    table switch per Exp->Ln transition).  Indices of the table list are
    act_func_set_ids, so ordering is preserved and other sets are
    emptied."""
    import concourse.bacc as bacc_mod
    from concourse.hw_specs import get_activation_tables as real_tabs

    def patched(arch):
        tabs = real_tabs(arch)
        return {name: (funcs if name == target else set())
                for name, funcs in tabs.items()}

    bacc_mod.get_activation_tables = patched


def _build():
    _force_act_table_set()
    nc = bacc.Bacc("TRN2", target_bir_lowering=False, debug=False,
                   num_devices=N_CORES, enable_asserts=False)
    xT = nc.declare_dram_parameter("xT", [128, KB], F32, isOutput=False)
    m0T = nc.declare_dram_parameter("m0T", [128, KO0], M_DT, isOutput=False)
    w0T = nc.declare_dram_parameter("w0T", [128, KO0], BF16, isOutput=False)
    m1T = nc.declare_dram_parameter("m1T", [128, KO1], M_DT, isOutput=False)
    w1T = nc.declare_dram_parameter("w1T", [128, KO1], BF16, isOutput=False)
    hd0T = nc.declare_dram_parameter("hd0T", [128, C], F32, isOutput=False)
    hd1T = nc.declare_dram_parameter("hd1T", [128, C], F32, isOutput=False)
    y_part = nc.declare_dram_parameter("y_part", [C, B], F32, isOutput=True)
    Act = mybir.ActivationFunctionType

    with tile.TileContext(nc) as tc:
        with (
            tc.tile_pool(name="sb", bufs=1) as sb,
            tc.tile_pool(name="ps", bufs=1, space="PSUM") as ps,
        ):
            # ---------------- SBUF tiles
            xf = sb.tile([128, KB], F32)
            xb = sb.tile([128, KB], BF16)
            m0 = sb.tile([128, KO0], M_DT)
            w0 = sb.tile([128, KO0], BF16)
            m1 = sb.tile([128, KO1], M_DT)
            w1 = sb.tile([128, KO1], BF16)
            hd0 = sb.tile([128, C], F32)
            hd1 = sb.tile([128, C], F32)
            eps = sb.tile([128, 1], F32)
            warm_i = sb.tile([128, 1], F32)
            warm_o = sb.tile([128, 1], F32)
            a0 = sb.tile([128, KB], F32)
            z0 = sb.tile([128, KB], F32)
            p0 = sb.tile([128, KB], F32)
            q0 = sb.tile([128, KB], F32)
            L0f = sb.tile([128, KB], F32)
            L0 = sb.tile([128, KB], BF16)
            d0 = sb.tile([128, KB], F32)
            h0f = sb.tile([128, KB], F32)
            h0b = sb.tile([128, KB], BF16)
            z1 = sb.tile([128, KB], F32)
            p1 = sb.tile([128, KB], F32)
            q1 = sb.tile([128, KB], F32)
            L1f = sb.tile([128, KB], F32)
            L1 = sb.tile([128, KB], BF16)
            d1 = sb.tile([128, B], F32)
            h1f = sb.tile([128, B], F32)
            yo = sb.tile([C, B], F32)
            # ---------------- PSUM
            S0p = ps.tile([128, KB], F32)
            HL0p = ps.tile([128, KB], F32)
            S1p = ps.tile([128, B], F32)
            HL1p = ps.tile([128, B], F32)
            Yp = ps.tile([C, B], F32)

            # ---------------- DMA issue: gpsimd takes the m-stream, sync
            # the w-stream.  Few, WIDE calls: descriptor count per call is
            # the 128 partition rows regardless of width, so wide rows
            # amortize both the ~10ns/row issue cost and the ~60ns/desc
            # ring overhead.  Halves (4096 cols) keep a 2-stage pipeline.
            prev = {"g": None, "s": None}

            def dma(eng, key, dst, src):
                op = eng.dma_start(dst, src)
                if prev[key] is not None:
                    _order(op, prev[key], f"{key}-dma order")
                prev[key] = op
                return op

            # block boundaries in ko units: coarse first (max ring
            # throughput), fine at the end (short trailing-chain tail)
            BLKS = [(0, 4), (4, 6), (6, 7), (7, 8)]
            d_x = dma(nc.sync, "s", xf[:], xT[:, :])
            d_m1 = dma(nc.gpsimd, "g", m1[:], m1T[:, :])
            d_w1 = dma(nc.sync, "s", w1[:], w1T[:, :])
            d_h0 = dma(nc.gpsimd, "g", hd0[:], hd0T[:, :])
            d_h1 = dma(nc.sync, "s", hd1[:], hd1T[:, :])
            for lo, hi in BLKS:
                sl = slice(lo * 1024, hi * 1024)
                dma(nc.gpsimd, "g", m0[:, sl], m0T[:, sl])
                dma(nc.sync, "s", w0[:, sl], w0T[:, sl])

            # ---------------- constants + act-table warm (forces the one
            # table load to run at kernel start, overlapping the x DMA)
            nc.vector.memset(eps[:], -1e-6)
            nc.vector.memset(warm_i[:], 0.0)
            warm = nc.scalar.activation(warm_o[:], warm_i[:], Act.Exp)

            # in-order engine queues: chain every op on its engine
            tail = {"sc": warm, "ve": None, "pe": None}

            def q(key, op):
                if tail[key] is not None:
                    _order(op, tail[key], f"{key} queue order")
                tail[key] = op
                return op

            def mm(out_ap, lhs_ap, rhs_ap, start, stop, skip=False):
                return q("pe", nc.tensor.matmul(out_ap, lhs_ap, rhs_ap,
                                                start=start, stop=stop,
                                                skip_group_check=skip))

            # ---------------- layer-0 L chain (x is small; single shot)
            # a0 = |x|; z0 = exp(-2g*a0 - 1e-6); L0 = max(ln(1-z0), CL) - ln(1+z0)
            q("ve", nc.vector.tensor_copy(xb[:], xf[:]))
            q("ve", nc.vector.scalar_tensor_tensor(
                a0[:], xf[:], -1.0, xf[:],
                op0=AluOpType.mult, op1=AluOpType.max))
            q("sc", nc.scalar.activation(z0[:], a0[:], Act.Exp,
                                         scale=-2.0 * GAMMA, bias=eps[:]))
            q("sc", nc.scalar.activation(p0[:], z0[:], Act.Ln,
                                         bias=1.0, scale=-1.0))
            q("sc", nc.scalar.activation(q0[:], z0[:], Act.Ln,
                                         bias=1.0, scale=1.0))
            q("ve", nc.vector.scalar_tensor_tensor(
                L0f[:], p0[:], LCLAMP, q0[:],
                op0=AluOpType.max, op1=AluOpType.subtract))
            q("ve", nc.vector.tensor_copy(L0[:], L0f[:]))

            # ---------------- layer-0 matmul sweep + trailing chains
            # PE runs the full S0/HL0 sweep uninterrupted (layer-1 matmuls
            # wait until the end; they are only ~1.1us and would stall the
            # in-order PE queue mid-stream waiting on the chains).  The
            # delta0/h0/L1 chains trail in 2-chunk steps on Scalar/Vector.
            def chain_step(lo, hi):            # chunks [lo, hi)
                bs = slice(lo * B, hi * B)
                q("sc", nc.scalar.activation(d0[:, bs], S0p[:, bs], Act.Exp))
                q("ve", nc.vector.scalar_tensor_tensor(
                    h0f[:, bs], HL0p[:, bs], 0.0, d0[:, bs],
                    op0=AluOpType.max, op1=AluOpType.mult))
                q("ve", nc.vector.tensor_copy(h0b[:, bs], h0f[:, bs]))
                q("sc", nc.scalar.activation(z1[:, bs], h0f[:, bs], Act.Exp,
                                             scale=-2.0 * GAMMA, bias=eps[:]))
                q("sc", nc.scalar.activation(p1[:, bs], z1[:, bs], Act.Ln,
                                             bias=1.0, scale=-1.0))
                q("sc", nc.scalar.activation(q1[:, bs], z1[:, bs], Act.Ln,
                                             bias=1.0, scale=1.0))
                q("ve", nc.vector.scalar_tensor_tensor(
                    L1f[:, bs], p1[:, bs], LCLAMP, q1[:, bs],
                    op0=AluOpType.max, op1=AluOpType.subtract))
                q("ve", nc.vector.tensor_copy(L1[:, bs], L1f[:, bs]))

            for ko in range(KCH):
                bs = slice(ko * B, (ko + 1) * B)
                for ki in range(KCH):
                    lsl = slice(ko * 1024 + ki * 128, ko * 1024 + (ki + 1) * 128)
                    rsl = slice(ki * B, (ki + 1) * B)
                    mm(S0p[:, bs], m0[:, lsl], L0[:, rsl],
                       start=(ki == 0), stop=(ki == KCH - 1))
                for ki in range(KCH):
                    lsl = slice(ko * 1024 + ki * 128, ko * 1024 + (ki + 1) * 128)
                    rsl = slice(ki * B, (ki + 1) * B)
                    mm(HL0p[:, bs], w0[:, lsl], xb[:, rsl],
                       start=(ki == 0), stop=(ki == KCH - 1))
                # 2-chunk steps early (fewer cross-engine hops), 1-chunk
                # steps at the end (start trailing work asap)
                if ko in (1, 3, 5):
                    chain_step(ko - 1, ko + 1)
                elif ko >= 6:
                    chain_step(ko, ko + 1)

            # ---------------- layer-1 + head matmuls, then finale
            for j in range(KCH):
                lsl = slice(j * 128, (j + 1) * 128)
                bsj = slice(j * B, (j + 1) * B)
                mm(S1p[:], m1[:, lsl], L1[:, bsj],
                   start=(j == 0), stop=(j == KCH - 1), skip=True)
                mm(HL1p[:], w1[:, lsl], h0b[:, bsj],
                   start=(j == 0), stop=(j == KCH - 1), skip=True)
            # head0 partial for this core's own chunk (slot 0)
            mm(Yp[:], hd0[:, :], h0f[:, 0:B], start=True, stop=False, skip=True)
            q("sc", nc.scalar.activation(d1[:], S1p[:], Act.Exp))
            q("ve", nc.vector.scalar_tensor_tensor(
                h1f[:], HL1p[:], 0.0, d1[:],
                op0=AluOpType.max, op1=AluOpType.mult))
            mm(Yp[:], hd1[:, :], h1f[:], start=False, stop=True, skip=True)
            q("ve", nc.vector.tensor_copy(yo[:], Yp[:]))
            dma(nc.sync, "s", y_part[:, :], yo[:])
    nc.compile()
    return nc


def _prep(x, v0, fc0, head0, v1, fc1, head1):
    """Host-side weight preprocessing -> per-core in_maps.

    Per core c, output-chunk slot k holds layer-0 output chunk
    (c+k)%8; layer-1 contraction slot k is rotated identically."""
    m0 = (np.asarray(v0) > 0).astype(np.float32)
    w0 = np.asarray(fc0, np.float32) * m0
    m1 = (np.asarray(v1) > 0).astype(np.float32)
    w1 = np.asarray(fc1, np.float32) * m1
    xT = np.asarray(x, np.float32).T                      # [1024, 64]
    xc = np.ascontiguousarray(
        xT.reshape(KCH, 128, B).transpose(1, 0, 2).reshape(128, KB))
    # [ki, ip, oc, op] blocks of the transposed layer-0 weights
    m0blk = m0.T.reshape(KCH, 128, KCH, 128)
    w0blk = w0.T.reshape(KCH, 128, KCH, 128)
    head0 = np.asarray(head0, np.float32)
    head1 = np.asarray(head1, np.float32)
    in_maps = []
    for c in range(N_CORES):
        perm = [(c + k) % KCH for k in range(KCH)]
        # slot-major k, then ki, then op: [ip, k, ki, op] -> [128, 8192]
        m0c = np.ascontiguousarray(
            m0blk[:, :, perm, :].transpose(1, 2, 0, 3).reshape(128, KO0)
        ).astype(M_NP)
        w0c = np.ascontiguousarray(
            w0blk[:, :, perm, :].transpose(1, 2, 0, 3).reshape(128, KO0)
        ).astype(BF16_NP)
        sl = slice(c * OSH, (c + 1) * OSH)
        m1t = m1[sl].T.reshape(KCH, 128, OSH)             # [ic, ip, o]
        w1t = w1[sl].T.reshape(KCH, 128, OSH)
        m1c = np.ascontiguousarray(
            m1t[perm].transpose(1, 0, 2).reshape(128, KO1)).astype(M_NP)
        w1c = np.ascontiguousarray(
            w1t[perm].transpose(1, 0, 2).reshape(128, KO1)).astype(BF16_NP)
        in_maps.append({
            "xT": xc,
            "m0T": m0c,
            "w0T": w0c,
            "m1T": m1c,
            "w1T": w1c,
            "hd0T": np.ascontiguousarray(head0[:, sl].T),
            "hd1T": np.ascontiguousarray(head1[:, sl].T),
        })
    return in_maps


def kernel(x, v0, fc0, head0, v1, fc1, head1):
    nc = _CACHE.get("nc")
    if nc is None:
        nc = _CACHE["nc"] = _build()
    in_maps = _prep(x, v0, fc0, head0, v1, fc1, head1)
    kwargs = {}
    if PROFILE["enable"]:
        kwargs = {"trace": True, **PROFILE["trace_kwargs"]}
    res = run_bass_kernel_spmd(nc, in_maps, core_ids=list(range(N_CORES)),
                               **kwargs)
    if PROFILE["enable"]:
        PROFILE["runs"].append(res)
    y = np.zeros((C, B), np.float32)
    for c in range(N_CORES):
        y += res.results[c]["y_part"]
    return np.ascontiguousarray(y.T).astype(np.float32)
